# revision 11
# baseline (speedup 1.0000x reference)
"""Trainium2 Bass kernel: causal multi-head self-attention.

Problem: B=2, T=4096, C=768, H=12, D=64, causal softmax(QK^T/sqrt(D))V + out proj.

Sharding (8 cores): core c handles batch b=c//4 and 3 heads g=c%4 (rows
192*g:192*(g+1) of wq/wk/wv, same columns of wo). Each core computes its
heads' full attention and a partial out-projection (T, C) for its batch;
the host sums the 4 partials per batch and transposes back to (B, T, C).

v3 highlights (on top of the v2 proj/attention software pipeline):
  - x and all weights stream in as bf16 (half the DMA traffic; matmul rate
    on TRN2 is column-count-bound, so bf16 operands cost the same cycles
    but less bandwidth/weight-load time)
  - Q2/K2 projections merged into one matmul via a host-packed [wq2|wk2]
    stationary block; K2 lands at partitions 64:128 and is shifted to
    KT2[0:64] with a local SBUF->SBUF DMA (engines cannot cross partitions)
  - V projection at its true 192 columns (bf16 moving operand has no
    min-free-size penalty, unlike float32r)
  - diagonal column pruning: for the diagonal l-tile at depth d, columns
    0..128d of the q-block are fully masked, so the scores matmul, the mask
    pre-accumulation (one shared 128x128 triangle), and the ctx matmul all
    skip them. exp still covers the full group; the pruned garbage columns
    are never read.
  - masks/identity/ones constants precomputed on host, DMA'd once (the
    GpSimd affine_select chain used to serialize 20us of startup)
  - PSUM: sp pool 2x3 banks shared by projection tiles and score groups,
    1 bank ctx accumulator, 1 bank out-proj staging

Attention operands bf16, PSUM accumulation fp32, ctx/out in f32r.
"""

import os
import sys
import types

import numpy as np
import ml_dtypes

if "/opt/trn_rl_repo" not in sys.path:
    sys.path.insert(0, "/opt/trn_rl_repo")

import concourse.bass as bass  # noqa: E402
import concourse.mybir as mybir  # noqa: E402
from concourse import bacc, tile  # noqa: E402
from concourse.bass_utils import run_bass_kernel_spmd  # noqa: E402

F32 = mybir.dt.float32
F32R = mybir.dt.float32r
BF16 = mybir.dt.bfloat16
EXP = mybir.ActivationFunctionType.Exp

B, T, C, H, D = 2, 4096, 768, 12, 64
HPD = 3          # heads per device
DH = HPD * D     # 192 local head channels
NCORES = 8
QB = 512         # query block (matmul free dim / PSUM bank)
LT = 128         # key(l)-tile size
GRP = 3          # l-tiles per exp group (3 PSUM banks)
BF = ml_dtypes.bfloat16


def build_kernel(t=T, trace_sim=False):
    n_lt = t // LT
    n_ch = t // QB
    nct = C // 128            # 6
    ndg = QB // LT            # diagonal tiles per q-block (4)

    nc = bacc.Bacc("TRN2", target_bir_lowering=False, debug=False,
                   num_devices=NCORES)
    xT_d = nc.dram_tensor("xT", [C, t], BF16, kind="ExternalInput")
    # packed QK weights: [wq01(128) | wk01(128) | wq2(64)|wk2(64)]
    wqkT_d = nc.dram_tensor("wqkT", [C, 384], BF16, kind="ExternalInput")
    wvT_d = nc.dram_tensor("wvT", [C, DH], BF16, kind="ExternalInput")
    woT_d = nc.dram_tensor("woT", [256, C], BF16, kind="ExternalInput")  # padded
    # constants: [tri(128) | ident(128) | ones(96)]
    cst_d = nc.dram_tensor("cst", [128, 352], BF16, kind="ExternalInput")
    outT_d = nc.dram_tensor("outT", [C, t], BF16, kind="ExternalOutput")

    with tile.TileContext(nc, trace_sim=trace_sim) as tc:
        with (
            tc.tile_pool(name="const", bufs=1) as const,
            tc.tile_pool(name="epool", bufs=3) as epool,
            tc.tile_pool(name="small", bufs=3) as small,
            tc.tile_pool(name="k2s", bufs=2) as k2s,
            tc.tile_pool(name="sp", bufs=2, space="PSUM") as sp,
            tc.tile_pool(name="cp", bufs=1, space="PSUM") as cp,
            tc.tile_pool(name="po", bufs=1, space="PSUM") as po_pool,
        ):
            # ---- x + weights: everything prefetched up front -------------
            # whole x lives in SBUF; per-(ct, chunk) DMAs give the
            # projection fine-grained dependencies while the DMA engines
            # stream the full tensor without per-chunk JIT pressure
            xall = const.tile([128, nct, t], BF16)
            xT_r = xT_d.ap().rearrange("(ct p) t -> p ct t", p=128)
            wqkT_s = const.tile([128, nct, 384], BF16)
            wvT_s = const.tile([128, nct, DH], BF16)
            cst_s = const.tile([128, 352], BF16)
            wqk_r = wqkT_d.ap().rearrange("(ct p) d -> p ct d", p=128)
            # interleave the first chunk's x and QK weights per-ct so the
            # first projection matmuls start after ~1/6 of the transfers
            for ct in range(nct):
                nc.sync.dma_start(wqkT_s[:, ct, :], wqk_r[:, ct, :])
                nc.sync.dma_start(xall[:, ct, 0:QB], xT_r[:, ct, 0:QB])
            nc.sync.dma_start(wvT_s[:], wvT_d.ap().rearrange("(ct p) d -> p ct d", p=128))
            nc.sync.dma_start(cst_s[:], cst_d.ap())
            for chp in range(1, n_ch):
                for ct in range(nct):
                    nc.sync.dma_start(
                        xall[:, ct, chp * QB:(chp + 1) * QB],
                        xT_r[:, ct, chp * QB:(chp + 1) * QB])
            woT_a = const.tile([128, C], BF16)
            woT_b = const.tile([128, C], BF16)   # rows 64:128 are host zeros
            nc.sync.dma_start(woT_a[:], woT_d.ap()[0:128, :])
            nc.sync.dma_start(woT_b[:], woT_d.ap()[128:256, :])

            tri = cst_s[:, 0:128]       # 0 if p <= f else -1e30
            identb = cst_s[:, 128:256]
            onesb = cst_s[:, 256:352]

            ones1 = const.tile([128, 1], F32)
            nc.vector.memset(ones1[:], 1.0)
            zero1 = const.tile([128, 1], F32)
            nc.vector.memset(zero1[:], 0.0)

            # ---- big persistent activations ------------------------------
            KT01 = const.tile([128, t], BF16)
            KT2 = const.tile([128, t], BF16)
            QTz = [const.tile([128, t], BF16, tag=f"qtz{h}", name=f"qtz{h}")
                   for h in range(HPD)]
            Vone = const.tile([128, n_lt, HPD * 65], BF16)
            ctxT01 = const.tile([128, t], BF16)
            ctxT2 = const.tile([128, t], BF16)   # rows 64:128 zeroed

            # zero-fill dead rows (avoids NaN*0 in the PE); QTz/KT2 first —
            # attention qb=0 needs them
            for buf in (*QTz, KT2):
                nc.vector.tensor_copy(buf[:], zero1[:].to_broadcast((128, t)))
            nc.vector.tensor_copy(
                Vone[:].rearrange("p l (h e) -> p l h e", e=65)[:, :, :, 64:65],
                onesb[:])
            nc.vector.tensor_copy(
                ctxT2[64:128, :], zero1[0:64, :].to_broadcast((64, t)))

            def emit_outproj(qb, oc, pool):
                qs = slice(qb * QB, (qb + 1) * QB)
                ocs = slice(oc * 128, (oc + 1) * 128)
                po = pool.tile([128, 3 * QB] if pool is sp else [128, QB],
                               F32, tag="sp" if pool is sp else "po")
                nc.tensor.matmul(po[:, 0:QB], woT_a[:, ocs], ctxT01[:, qs],
                                 start=True, stop=False)
                nc.tensor.matmul(po[:, 0:QB], woT_b[:, ocs], ctxT2[:, qs],
                                 start=False, stop=True)
                ot = small.tile([128, QB], BF16, tag="ot")
                nc.vector.tensor_copy(ot[:], po[:, 0:QB])
                nc.sync.dma_start(outT_d.ap()[ocs, qs], ot[:])

            pending = []
            for ch in range(n_ch):
                cs = slice(ch * QB, (ch + 1) * QB)
                # ---- projection of chunk ch ------------------------------
                xc = xall[:, :, cs]

                t1 = sp.tile([128, 3 * QB], F32, tag="sp")
                for ct in range(nct):
                    f, l = (ct == 0), (ct == nct - 1)
                    nc.tensor.matmul(t1[:, 0:QB], wqkT_s[:, ct, 0:128],
                                     xc[:, ct, :], start=f, stop=l)
                    nc.tensor.matmul(t1[:, QB:2 * QB], wqkT_s[:, ct, 128:256],
                                     xc[:, ct, :], start=f, stop=l)
                    nc.tensor.matmul(t1[:, 2 * QB:3 * QB], wqkT_s[:, ct, 256:384],
                                     xc[:, ct, :], start=f, stop=l)
                t2 = sp.tile([128, 3 * QB], F32, tag="sp")
                for ts in range(QB // 128):
                    # 256-stride placement keeps each 192-col output inside
                    # one PSUM bank
                    pv = t2[:, ts * 256:ts * 256 + DH]
                    for ct in range(nct):
                        nc.tensor.matmul(pv, xc[:, ct, ts * 128:(ts + 1) * 128],
                                         wvT_s[:, ct, :], start=(ct == 0),
                                         stop=(ct == nct - 1))

                # copy projections out of PSUM (lane-aligned except K2)
                nc.vector.tensor_copy(QTz[0][0:64, cs], t1[0:64, 0:QB])
                nc.vector.tensor_copy(QTz[1][64:128, cs], t1[64:128, 0:QB])
                nc.vector.tensor_copy(KT01[:, cs], t1[:, QB:2 * QB])
                nc.vector.tensor_copy(QTz[2][0:64, cs], t1[0:64, 2 * QB:3 * QB])
                # K2 sits at partitions 64:128; stage + local DMA shifts it
                # down to KT2[0:64] (engines cannot move data across lanes)
                k2t = k2s.tile([128, QB], BF16, tag="k2t")
                nc.vector.tensor_copy(k2t[64:128, :], t1[64:128, 2 * QB:3 * QB])
                nc.sync.dma_start(KT2[0:64, cs], k2t[64:128, :])
                for ts in range(QB // 128):
                    tt = ch * (QB // 128) + ts
                    nc.vector.tensor_copy(
                        Vone[:, tt, :].rearrange("p (h e) -> p h e", e=65)[:, :, 0:64],
                        t2[:, ts * 256:ts * 256 + DH]
                          .rearrange("p (h e) -> p h e", e=64))

                # ---- attention q-block qb = ch ---------------------------
                qb = ch
                qs = cs
                L = (qb + 1) * ndg
                def normalize(h, ctxp):
                    # free the PSUM accumulator fast, normalize off-path
                    stg = small.tile([128, QB], F32, tag="stg")
                    nc.vector.tensor_copy(stg[0:65, :], ctxp[0:65, :])
                    dn = small.tile([1, QB], F32, tag="dn")
                    nc.vector.tensor_copy(dn[:], stg[64:65, :])
                    rec = small.tile([1, QB], F32, tag="rec")
                    nc.vector.reciprocal_approx_fast(rec[:], dn[:])
                    rb = small.tile([64, QB], F32, tag="rb")
                    nc.gpsimd.partition_broadcast(rb[:], rec[:])
                    if h == 1:
                        st2 = small.tile([64, QB], BF16, tag="st2")
                        nc.vector.tensor_mul(st2[:], stg[0:64, :], rb[:])
                        nc.sync.dma_start(ctxT01[64:128, qs], st2[:])
                    else:
                        dst = ctxT01 if h == 0 else ctxT2
                        nc.vector.tensor_mul(dst[0:64, qs], stg[0:64, :], rb[:])

                def emit_ctx(item):
                    h, g0, gl, et, ctxp, last = item
                    for i in range(gl):
                        lt = g0 + i
                        d = lt - qb * ndg
                        q0 = max(0, d) * LT
                        nc.tensor.matmul(ctxp[:, q0:QB],
                                         Vone[:, lt, h * 65:h * 65 + 65],
                                         et[:, i * QB + q0:(i + 1) * QB],
                                         start=(lt == 0), stop=(lt == L - 1),
                                         skip_group_check=True)
                    if last:
                        normalize(h, ctxp)

                # ctx groups are deferred one iteration so exp(g+1) never
                # transitively waits on ctx(g) through the PE completion
                # counter (the scores->exp->ctx lockstep in flat order)
                prev_ctx = None
                for h in range(HPD):
                    KT_h = KT01 if h < 2 else KT2
                    ctxp = cp.tile([65, QB], F32, tag="cp")
                    for g0 in range(0, L, GRP):
                        gl = min(GRP, L - g0)
                        spt = sp.tile([128, 3 * QB], F32, tag="sp")
                        for i in range(gl):
                            lt = g0 + i
                            d = lt - qb * ndg
                            kt = KT_h[:, lt * LT:(lt + 1) * LT]
                            if d < 0:
                                nc.tensor.matmul(spt[:, i * QB:(i + 1) * QB],
                                                 kt, QTz[h][:, qs],
                                                 start=True, stop=True)
                            else:
                                # cols 0..128d fully masked -> pruned.
                                # strip [q0, q0+128): triangle mask pre-acc
                                # + scores; beyond: plain scores.
                                q0 = d * LT
                                strip = spt[:, i * QB + q0:i * QB + q0 + LT]
                                nc.tensor.matmul(strip, identb, tri,
                                                 start=True, stop=False)
                                nc.tensor.matmul(
                                    strip, kt,
                                    QTz[h][:, qb * QB + q0:qb * QB + q0 + LT],
                                    start=False, stop=True)
                                if q0 + LT < QB:
                                    nc.tensor.matmul(
                                        spt[:, i * QB + q0 + LT:(i + 1) * QB],
                                        kt,
                                        QTz[h][:, qb * QB + q0 + LT:(qb + 1) * QB],
                                        start=True, stop=True)
                        et = epool.tile([128, GRP * QB], BF16)
                        nc.scalar.activation(et[:, :gl * QB], spt[:, :gl * QB],
                                             EXP, scale=0.125)
                        if prev_ctx is not None:
                            emit_ctx(prev_ctx)
                        prev_ctx = (h, g0, gl, et, ctxp, g0 + gl >= L)
                        if pending:
                            emit_outproj(*pending.pop(0), po_pool)
                emit_ctx(prev_ctx)
                pending.extend((qb, oc) for oc in range(nct))
            # tail: alternate the po bank and a free sp buffer so the last
            # six emissions overlap
            for n, item in enumerate(pending):
                emit_outproj(*item, po_pool if n % 2 == 0 else sp)

    nc.compile()
    return nc


_NC_CACHE = {}
LAST_EXEC_NS = None
LAST_RES = None


def _get_nc():
    if "full" not in _NC_CACHE:
        _NC_CACHE["full"] = build_kernel()
    return _NC_CACHE["full"]


def _install_ntff_shim():
    """Make run_bass_kernel_spmd(trace=True) work under axon in this image."""
    import antenv
    if "antenv.axon_hooks" in sys.modules:
        return
    mod = types.ModuleType("antenv.axon_hooks")
    mod._hook = None
    mod.set_axon_ntff_profile_hook = lambda h: setattr(mod, "_hook", h)
    mod.get_axon_ntff_profile_hook = lambda: mod._hook
    sys.modules["antenv.axon_hooks"] = mod
    antenv.axon_hooks = mod
    try:
        from trn_agent_boot.trn_boot import _ntff_profile_via_ctypes
        mod.set_axon_ntff_profile_hook(
            _ntff_profile_via_ctypes("/opt/axon/libaxon_pjrt.so"))
    except Exception:
        pass


def make_in_maps(x, wq, wk, wv, wo):
    x = np.asarray(x, dtype=np.float32)
    wq = np.asarray(wq, dtype=np.float32)
    wk = np.asarray(wk, dtype=np.float32)
    wv = np.asarray(wv, dtype=np.float32)
    wo = np.asarray(wo, dtype=np.float32)

    # constants: triangle mask, identity, ones
    p = np.arange(128)[:, None]
    f = np.arange(128)[None, :]
    tri = np.where(p <= f, 0.0, -1.0e30).astype(np.float32)
    cst = np.concatenate(
        [tri, np.eye(128, dtype=np.float32), np.ones((128, 96), np.float32)],
        axis=1).astype(BF)

    in_maps = []
    for c in range(NCORES):
        b, g = c // (NCORES // B), c % (NCORES // B)
        rs, re = g * DH, (g + 1) * DH
        # packed stationary: [wq01 | wk01 | wq2|wk2] (transposed)
        wqk = np.concatenate([
            wq[rs:rs + 128].T, wk[rs:rs + 128].T,
            wq[rs + 128:re].T, wk[rs + 128:re].T], axis=1)
        woT = np.zeros((256, C), dtype=np.float32)
        woT[:DH] = wo[:, rs:re].T
        in_maps.append({
            "xT": np.ascontiguousarray(x[b].T).astype(BF),
            "wqkT": np.ascontiguousarray(wqk).astype(BF),
            "wvT": np.ascontiguousarray(wv[rs:re].T).astype(BF),
            "woT": woT.astype(BF),
            "cst": cst,
        })
    return in_maps


def kernel(x, wq, wk, wv, wo):
    global LAST_EXEC_NS, LAST_RES
    in_maps = make_in_maps(x, wq, wk, wv, wo)
    nc = _get_nc()
    trace = bool(int(os.environ.get("KERNEL_TRACE", "0")))
    if trace:
        try:
            _install_ntff_shim()
        except Exception:
            trace = False
    try:
        res = run_bass_kernel_spmd(nc, in_maps, core_ids=list(range(NCORES)),
                                   trace=trace)
    except Exception:
        if not trace:
            raise
        res = run_bass_kernel_spmd(nc, in_maps, core_ids=list(range(NCORES)),
                                   trace=False)
    LAST_EXEC_NS = res.exec_time_ns
    LAST_RES = res
    outT = [res.results[c]["outT"] for c in range(NCORES)]
    halves = []
    for b in range(B):
        acc = outT[4 * b].astype(np.float64)
        for c in range(4 * b + 1, 4 * b + 4):
            acc = acc + outT[c]
        halves.append(acc.T)
    return np.stack(halves).astype(np.float32)


# revision 12
# speedup vs baseline: 1.0232x; 1.0232x over previous
"""Trainium2 Bass kernel: causal multi-head self-attention.

Problem: B=2, T=4096, C=768, H=12, D=64, causal softmax(QK^T/sqrt(D))V + out proj.

Sharding (8 cores): core c handles batch b=c//4 and 3 heads g=c%4 (rows
192*g:192*(g+1) of wq/wk/wv, same columns of wo). Each core computes its
heads' full attention and a partial out-projection (T, C) for its batch;
the host sums the 4 partials per batch and transposes back to (B, T, C).

v3 highlights (on top of the v2 proj/attention software pipeline):
  - x and all weights stream in as bf16 (half the DMA traffic; matmul rate
    on TRN2 is column-count-bound, so bf16 operands cost the same cycles
    but less bandwidth/weight-load time)
  - Q2/K2 projections merged into one matmul via a host-packed [wq2|wk2]
    stationary block; K2 lands at partitions 64:128 and is shifted to
    KT2[0:64] with a local SBUF->SBUF DMA (engines cannot cross partitions)
  - V projection at its true 192 columns (bf16 moving operand has no
    min-free-size penalty, unlike float32r)
  - diagonal column pruning: for the diagonal l-tile at depth d, columns
    0..128d of the q-block are fully masked, so the scores matmul, the mask
    pre-accumulation (one shared 128x128 triangle), and the ctx matmul all
    skip them. exp still covers the full group; the pruned garbage columns
    are never read.
  - masks/identity/ones constants precomputed on host, DMA'd once (the
    GpSimd affine_select chain used to serialize 20us of startup)
  - PSUM: sp pool 2x3 banks shared by projection tiles and score groups,
    1 bank ctx accumulator, 1 bank out-proj staging

Attention operands bf16, PSUM accumulation fp32, ctx/out in f32r.
"""

import os
import sys
import types

import numpy as np
import ml_dtypes

if "/opt/trn_rl_repo" not in sys.path:
    sys.path.insert(0, "/opt/trn_rl_repo")

import concourse.bass as bass  # noqa: E402
import concourse.mybir as mybir  # noqa: E402
from concourse import bacc, tile  # noqa: E402
from concourse.bass_utils import run_bass_kernel_spmd  # noqa: E402

F32 = mybir.dt.float32
F32R = mybir.dt.float32r
BF16 = mybir.dt.bfloat16
EXP = mybir.ActivationFunctionType.Exp

B, T, C, H, D = 2, 4096, 768, 12, 64
HPD = 3          # heads per device
DH = HPD * D     # 192 local head channels
NCORES = 8
QB = 512         # query block (matmul free dim / PSUM bank)
LT = 128         # key(l)-tile size
GRP = 3          # l-tiles per exp group (3 PSUM banks)
BF = ml_dtypes.bfloat16


def build_kernel(t=T, trace_sim=False):
    n_lt = t // LT
    n_ch = t // QB
    nct = C // 128            # 6
    ndg = QB // LT            # diagonal tiles per q-block (4)

    nc = bacc.Bacc("TRN2", target_bir_lowering=False, debug=False,
                   num_devices=NCORES)
    xT_d = nc.dram_tensor("xT", [C, t], BF16, kind="ExternalInput")
    # packed QK weights: [wq01(128) | wk01(128) | wq2(64)|wk2(64)]
    wqkT_d = nc.dram_tensor("wqkT", [C, 384], BF16, kind="ExternalInput")
    wvT_d = nc.dram_tensor("wvT", [C, DH], BF16, kind="ExternalInput")
    woT_d = nc.dram_tensor("woT", [256, C], BF16, kind="ExternalInput")  # padded
    # constants: [tri(128) | ident(128) | ones(96)]
    cst_d = nc.dram_tensor("cst", [128, 352], BF16, kind="ExternalInput")
    outT_d = nc.dram_tensor("outT", [C, t], BF16, kind="ExternalOutput")

    with tile.TileContext(nc, trace_sim=trace_sim) as tc:
        with (
            tc.tile_pool(name="const", bufs=1) as const,
            tc.tile_pool(name="epool", bufs=3) as epool,
            tc.tile_pool(name="small", bufs=3) as small,
            tc.tile_pool(name="k2s", bufs=2) as k2s,
            tc.tile_pool(name="sp", bufs=2, space="PSUM") as sp,
            tc.tile_pool(name="cp", bufs=1, space="PSUM") as cp,
            tc.tile_pool(name="po", bufs=1, space="PSUM") as po_pool,
        ):
            # ---- x + weights: everything prefetched up front -------------
            # whole x lives in SBUF; per-(ct, chunk) DMAs give the
            # projection fine-grained dependencies while the DMA engines
            # stream the full tensor without per-chunk JIT pressure
            xall = const.tile([128, nct, t], BF16)
            xT_r = xT_d.ap().rearrange("(ct p) t -> p ct t", p=128)
            wqkT_s = const.tile([128, nct, 384], BF16)
            wvT_s = const.tile([128, nct, DH], BF16)
            cst_s = const.tile([128, 352], BF16)
            wqk_r = wqkT_d.ap().rearrange("(ct p) d -> p ct d", p=128)
            # interleave the first chunk's x and QK weights per-ct so the
            # first projection matmuls start after ~1/6 of the transfers
            for ct in range(nct):
                nc.sync.dma_start(wqkT_s[:, ct, :], wqk_r[:, ct, :])
                nc.sync.dma_start(xall[:, ct, 0:QB], xT_r[:, ct, 0:QB])
            nc.sync.dma_start(wvT_s[:], wvT_d.ap().rearrange("(ct p) d -> p ct d", p=128))
            nc.sync.dma_start(cst_s[:], cst_d.ap())
            def fetch_x(chp):
                if chp < n_ch:
                    for ct in range(nct):
                        nc.sync.dma_start(
                            xall[:, ct, chp * QB:(chp + 1) * QB],
                            xT_r[:, ct, chp * QB:(chp + 1) * QB])
            fetch_x(1)
            fetch_x(2)
            woT_a = const.tile([128, C], BF16)
            woT_b = const.tile([128, C], BF16)   # rows 64:128 are host zeros
            nc.sync.dma_start(woT_a[:], woT_d.ap()[0:128, :])
            nc.sync.dma_start(woT_b[:], woT_d.ap()[128:256, :])

            tri = cst_s[:, 0:128]       # 0 if p <= f else -1e30
            identb = cst_s[:, 128:256]
            onesb = cst_s[:, 256:352]

            ones1 = const.tile([128, 1], F32)
            nc.vector.memset(ones1[:], 1.0)
            zero1 = const.tile([128, 1], F32)
            nc.vector.memset(zero1[:], 0.0)

            # ---- big persistent activations ------------------------------
            KT01 = const.tile([128, t], BF16)
            KT2 = const.tile([128, t], BF16)
            QTz = [const.tile([128, t], BF16, tag=f"qtz{h}", name=f"qtz{h}")
                   for h in range(HPD)]
            Vone = const.tile([128, n_lt, HPD * 65], BF16)
            ctxT01 = const.tile([128, t], BF16)
            ctxT2 = const.tile([128, t], BF16)   # rows 64:128 zeroed

            # zero-fill dead rows (avoids NaN*0 in the PE); QTz/KT2 first —
            # attention qb=0 needs them
            for buf in (*QTz, KT2):
                nc.vector.tensor_copy(buf[:], zero1[:].to_broadcast((128, t)))
            nc.vector.tensor_copy(
                Vone[:].rearrange("p l (h e) -> p l h e", e=65)[:, :, :, 64:65],
                onesb[:])
            nc.vector.tensor_copy(
                ctxT2[64:128, :], zero1[0:64, :].to_broadcast((64, t)))

            def emit_outproj(qb, oc, pool):
                qs = slice(qb * QB, (qb + 1) * QB)
                ocs = slice(oc * 128, (oc + 1) * 128)
                po = pool.tile([128, 3 * QB] if pool is sp else [128, QB],
                               F32, tag="sp" if pool is sp else "po")
                nc.tensor.matmul(po[:, 0:QB], woT_a[:, ocs], ctxT01[:, qs],
                                 start=True, stop=False)
                nc.tensor.matmul(po[:, 0:QB], woT_b[:, ocs], ctxT2[:, qs],
                                 start=False, stop=True)
                ot = small.tile([128, QB], BF16, tag="ot")
                nc.vector.tensor_copy(ot[:], po[:, 0:QB])
                nc.sync.dma_start(outT_d.ap()[ocs, qs], ot[:])

            pending = []
            for ch in range(n_ch):
                cs = slice(ch * QB, (ch + 1) * QB)
                # ---- projection of chunk ch ------------------------------
                fetch_x(ch + 3)
                xc = xall[:, :, cs]

                t1 = sp.tile([128, 3 * QB], F32, tag="sp")
                for ct in range(nct):
                    f, l = (ct == 0), (ct == nct - 1)
                    nc.tensor.matmul(t1[:, 0:QB], wqkT_s[:, ct, 0:128],
                                     xc[:, ct, :], start=f, stop=l)
                    nc.tensor.matmul(t1[:, QB:2 * QB], wqkT_s[:, ct, 128:256],
                                     xc[:, ct, :], start=f, stop=l)
                    nc.tensor.matmul(t1[:, 2 * QB:3 * QB], wqkT_s[:, ct, 256:384],
                                     xc[:, ct, :], start=f, stop=l)
                t2 = sp.tile([128, 3 * QB], F32, tag="sp")
                for ts in range(QB // 128):
                    # 256-stride placement keeps each 192-col output inside
                    # one PSUM bank
                    pv = t2[:, ts * 256:ts * 256 + DH]
                    for ct in range(nct):
                        nc.tensor.matmul(pv, xc[:, ct, ts * 128:(ts + 1) * 128],
                                         wvT_s[:, ct, :], start=(ct == 0),
                                         stop=(ct == nct - 1))

                # copy projections out of PSUM (lane-aligned except K2)
                nc.vector.tensor_copy(QTz[0][0:64, cs], t1[0:64, 0:QB])
                nc.vector.tensor_copy(QTz[1][64:128, cs], t1[64:128, 0:QB])
                nc.vector.tensor_copy(KT01[:, cs], t1[:, QB:2 * QB])
                nc.vector.tensor_copy(QTz[2][0:64, cs], t1[0:64, 2 * QB:3 * QB])
                # K2 sits at partitions 64:128; stage + local DMA shifts it
                # down to KT2[0:64] (engines cannot move data across lanes)
                k2t = k2s.tile([128, QB], BF16, tag="k2t")
                nc.vector.tensor_copy(k2t[64:128, :], t1[64:128, 2 * QB:3 * QB])
                nc.gpsimd.dma_start(KT2[0:64, cs], k2t[64:128, :])
                for ts in range(QB // 128):
                    tt = ch * (QB // 128) + ts
                    nc.vector.tensor_copy(
                        Vone[:, tt, :].rearrange("p (h e) -> p h e", e=65)[:, :, 0:64],
                        t2[:, ts * 256:ts * 256 + DH]
                          .rearrange("p (h e) -> p h e", e=64))

                # ---- attention q-block qb = ch ---------------------------
                qb = ch
                qs = cs
                L = (qb + 1) * ndg
                def normalize(h, ctxp):
                    # free the PSUM accumulator fast, normalize off-path
                    stg = small.tile([128, QB], F32, tag="stg")
                    nc.vector.tensor_copy(stg[0:65, :], ctxp[0:65, :])
                    dn = small.tile([1, QB], F32, tag="dn")
                    nc.vector.tensor_copy(dn[:], stg[64:65, :])
                    rec = small.tile([1, QB], F32, tag="rec")
                    nc.vector.reciprocal_approx_fast(rec[:], dn[:])
                    rb = small.tile([64, QB], F32, tag="rb")
                    nc.gpsimd.partition_broadcast(rb[:], rec[:])
                    if h == 1:
                        st2 = small.tile([64, QB], BF16, tag="st2")
                        nc.vector.tensor_mul(st2[:], stg[0:64, :], rb[:])
                        nc.gpsimd.dma_start(ctxT01[64:128, qs], st2[:])
                    else:
                        dst = ctxT01 if h == 0 else ctxT2
                        nc.vector.tensor_mul(dst[0:64, qs], stg[0:64, :], rb[:])

                def emit_ctx(item):
                    h, g0, gl, et, ctxp, last = item
                    for i in range(gl):
                        lt = g0 + i
                        d = lt - qb * ndg
                        q0 = max(0, d) * LT
                        nc.tensor.matmul(ctxp[:, q0:QB],
                                         Vone[:, lt, h * 65:h * 65 + 65],
                                         et[:, i * QB + q0:(i + 1) * QB],
                                         start=(lt == 0), stop=(lt == L - 1),
                                         skip_group_check=True)
                    if last:
                        normalize(h, ctxp)

                # ctx groups are deferred one iteration so exp(g+1) never
                # transitively waits on ctx(g) through the PE completion
                # counter (the scores->exp->ctx lockstep in flat order)
                prev_ctx = None
                for h in range(HPD):
                    KT_h = KT01 if h < 2 else KT2
                    ctxp = cp.tile([65, QB], F32, tag="cp")
                    for g0 in range(0, L, GRP):
                        gl = min(GRP, L - g0)
                        spt = sp.tile([128, 3 * QB], F32, tag="sp")
                        for i in range(gl):
                            lt = g0 + i
                            d = lt - qb * ndg
                            kt = KT_h[:, lt * LT:(lt + 1) * LT]
                            if d < 0:
                                nc.tensor.matmul(spt[:, i * QB:(i + 1) * QB],
                                                 kt, QTz[h][:, qs],
                                                 start=True, stop=True)
                            else:
                                # cols 0..128d fully masked -> pruned.
                                # strip [q0, q0+128): triangle mask pre-acc
                                # + scores; beyond: plain scores.
                                q0 = d * LT
                                strip = spt[:, i * QB + q0:i * QB + q0 + LT]
                                nc.tensor.matmul(strip, identb, tri,
                                                 start=True, stop=False)
                                nc.tensor.matmul(
                                    strip, kt,
                                    QTz[h][:, qb * QB + q0:qb * QB + q0 + LT],
                                    start=False, stop=True)
                                if q0 + LT < QB:
                                    nc.tensor.matmul(
                                        spt[:, i * QB + q0 + LT:(i + 1) * QB],
                                        kt,
                                        QTz[h][:, qb * QB + q0 + LT:(qb + 1) * QB],
                                        start=True, stop=True)
                        et = epool.tile([128, GRP * QB], BF16)
                        nc.scalar.activation(et[:, :gl * QB], spt[:, :gl * QB],
                                             EXP, scale=0.125)
                        if prev_ctx is not None:
                            emit_ctx(prev_ctx)
                        prev_ctx = (h, g0, gl, et, ctxp, g0 + gl >= L)
                        if pending:
                            emit_outproj(*pending.pop(0), po_pool)
                emit_ctx(prev_ctx)
                pending.extend((qb, oc) for oc in range(nct))
            # tail: alternate the po bank and a free sp buffer so the last
            # six emissions overlap
            for n, item in enumerate(pending):
                emit_outproj(*item, po_pool if n % 2 == 0 else sp)

    nc.compile()
    return nc


_NC_CACHE = {}
LAST_EXEC_NS = None
LAST_RES = None


def _get_nc():
    if "full" not in _NC_CACHE:
        _NC_CACHE["full"] = build_kernel()
    return _NC_CACHE["full"]


def _install_ntff_shim():
    """Make run_bass_kernel_spmd(trace=True) work under axon in this image."""
    import antenv
    if "antenv.axon_hooks" in sys.modules:
        return
    mod = types.ModuleType("antenv.axon_hooks")
    mod._hook = None
    mod.set_axon_ntff_profile_hook = lambda h: setattr(mod, "_hook", h)
    mod.get_axon_ntff_profile_hook = lambda: mod._hook
    sys.modules["antenv.axon_hooks"] = mod
    antenv.axon_hooks = mod
    try:
        from trn_agent_boot.trn_boot import _ntff_profile_via_ctypes
        mod.set_axon_ntff_profile_hook(
            _ntff_profile_via_ctypes("/opt/axon/libaxon_pjrt.so"))
    except Exception:
        pass


def make_in_maps(x, wq, wk, wv, wo):
    x = np.asarray(x, dtype=np.float32)
    wq = np.asarray(wq, dtype=np.float32)
    wk = np.asarray(wk, dtype=np.float32)
    wv = np.asarray(wv, dtype=np.float32)
    wo = np.asarray(wo, dtype=np.float32)

    # constants: triangle mask, identity, ones
    p = np.arange(128)[:, None]
    f = np.arange(128)[None, :]
    tri = np.where(p <= f, 0.0, -1.0e30).astype(np.float32)
    cst = np.concatenate(
        [tri, np.eye(128, dtype=np.float32), np.ones((128, 96), np.float32)],
        axis=1).astype(BF)

    in_maps = []
    for c in range(NCORES):
        b, g = c // (NCORES // B), c % (NCORES // B)
        rs, re = g * DH, (g + 1) * DH
        # packed stationary: [wq01 | wk01 | wq2|wk2] (transposed)
        wqk = np.concatenate([
            wq[rs:rs + 128].T, wk[rs:rs + 128].T,
            wq[rs + 128:re].T, wk[rs + 128:re].T], axis=1)
        woT = np.zeros((256, C), dtype=np.float32)
        woT[:DH] = wo[:, rs:re].T
        in_maps.append({
            "xT": np.ascontiguousarray(x[b].T).astype(BF),
            "wqkT": np.ascontiguousarray(wqk).astype(BF),
            "wvT": np.ascontiguousarray(wv[rs:re].T).astype(BF),
            "woT": woT.astype(BF),
            "cst": cst,
        })
    return in_maps


def kernel(x, wq, wk, wv, wo):
    global LAST_EXEC_NS, LAST_RES
    in_maps = make_in_maps(x, wq, wk, wv, wo)
    nc = _get_nc()
    trace = bool(int(os.environ.get("KERNEL_TRACE", "0")))
    if trace:
        try:
            _install_ntff_shim()
        except Exception:
            trace = False
    try:
        res = run_bass_kernel_spmd(nc, in_maps, core_ids=list(range(NCORES)),
                                   trace=trace)
    except Exception:
        if not trace:
            raise
        res = run_bass_kernel_spmd(nc, in_maps, core_ids=list(range(NCORES)),
                                   trace=False)
    LAST_EXEC_NS = res.exec_time_ns
    LAST_RES = res
    outT = [res.results[c]["outT"] for c in range(NCORES)]
    halves = []
    for b in range(B):
        acc = outT[4 * b].astype(np.float64)
        for c in range(4 * b + 1, 4 * b + 4):
            acc = acc + outT[c]
        halves.append(acc.T)
    return np.stack(halves).astype(np.float32)


# revision 13
# speedup vs baseline: 1.0489x; 1.0251x over previous
"""Trainium2 Bass kernel: causal multi-head self-attention.

Problem: B=2, T=4096, C=768, H=12, D=64, causal softmax(QK^T/sqrt(D))V + out proj.

Sharding (8 cores): core c handles batch b=c//4 and 3 heads g=c%4 (rows
192*g:192*(g+1) of wq/wk/wv, same columns of wo). Each core computes its
heads' full attention and a partial out-projection (T, C) for its batch;
the host sums the 4 partials per batch and transposes back to (B, T, C).

v3 highlights (on top of the v2 proj/attention software pipeline):
  - x and all weights stream in as bf16 (half the DMA traffic; matmul rate
    on TRN2 is column-count-bound, so bf16 operands cost the same cycles
    but less bandwidth/weight-load time)
  - Q2/K2 projections merged into one matmul via a host-packed [wq2|wk2]
    stationary block; K2 lands at partitions 64:128 and is shifted to
    KT2[0:64] with a local SBUF->SBUF DMA (engines cannot cross partitions)
  - V projection at its true 192 columns (bf16 moving operand has no
    min-free-size penalty, unlike float32r)
  - diagonal column pruning: for the diagonal l-tile at depth d, columns
    0..128d of the q-block are fully masked, so the scores matmul, the mask
    pre-accumulation (one shared 128x128 triangle), and the ctx matmul all
    skip them. exp still covers the full group; the pruned garbage columns
    are never read.
  - masks/identity/ones constants precomputed on host, DMA'd once (the
    GpSimd affine_select chain used to serialize 20us of startup)
  - PSUM: sp pool 2x3 banks shared by projection tiles and score groups,
    1 bank ctx accumulator, 1 bank out-proj staging

Attention operands bf16, PSUM accumulation fp32, ctx/out in f32r.
"""

import os
import sys
import types

import numpy as np
import ml_dtypes

if "/opt/trn_rl_repo" not in sys.path:
    sys.path.insert(0, "/opt/trn_rl_repo")

import concourse.bass as bass  # noqa: E402
import concourse.mybir as mybir  # noqa: E402
from concourse import bacc, tile  # noqa: E402
from concourse.bass_utils import run_bass_kernel_spmd  # noqa: E402

F32 = mybir.dt.float32
F32R = mybir.dt.float32r
BF16 = mybir.dt.bfloat16
EXP = mybir.ActivationFunctionType.Exp

B, T, C, H, D = 2, 4096, 768, 12, 64
HPD = 3          # heads per device
DH = HPD * D     # 192 local head channels
NCORES = 8
QB = 512         # query block (matmul free dim / PSUM bank)
LT = 128         # key(l)-tile size
GRP = 3          # l-tiles per exp group (3 PSUM banks)
BF = ml_dtypes.bfloat16


def build_kernel(t=T, trace_sim=False):
    n_lt = t // LT
    n_ch = t // QB
    nct = C // 128            # 6
    ndg = QB // LT            # diagonal tiles per q-block (4)

    nc = bacc.Bacc("TRN2", target_bir_lowering=False, debug=False,
                   num_devices=NCORES)
    xT_d = nc.dram_tensor("xT", [C, t], BF16, kind="ExternalInput")
    # packed QK weights: [wq01(128) | wk01(128) | wq2(64)|wk2(64)]
    wqkT_d = nc.dram_tensor("wqkT", [C, 384], BF16, kind="ExternalInput")
    wvT_d = nc.dram_tensor("wvT", [C, DH], BF16, kind="ExternalInput")
    woT_d = nc.dram_tensor("woT", [256, C], BF16, kind="ExternalInput")  # padded
    # constants: [tri(128) | ident(128) | ones(96)]
    cst_d = nc.dram_tensor("cst", [128, 352], BF16, kind="ExternalInput")
    outT_d = nc.dram_tensor("outT", [C, t], BF16, kind="ExternalOutput")

    with tile.TileContext(nc, trace_sim=trace_sim) as tc:
        with (
            tc.tile_pool(name="const", bufs=1) as const,
            tc.tile_pool(name="epool", bufs=3) as epool,
            tc.tile_pool(name="small", bufs=3) as small,
            tc.tile_pool(name="k2s", bufs=2) as k2s,
            tc.tile_pool(name="sp", bufs=2, space="PSUM") as sp,
            tc.tile_pool(name="cp", bufs=1, space="PSUM") as cp,
            tc.tile_pool(name="po", bufs=1, space="PSUM") as po_pool,
        ):
            # ---- x + weights: everything prefetched up front -------------
            # whole x lives in SBUF; per-(ct, chunk) DMAs give the
            # projection fine-grained dependencies while the DMA engines
            # stream the full tensor without per-chunk JIT pressure
            xall = const.tile([128, nct, t], BF16)
            xT_r = xT_d.ap().rearrange("(ct p) t -> p ct t", p=128)
            wqkT_s = const.tile([128, nct, 384], BF16)
            wvT_s = const.tile([128, nct, DH], BF16)
            cst_s = const.tile([128, 352], BF16)
            wqk_r = wqkT_d.ap().rearrange("(ct p) d -> p ct d", p=128)
            # interleave the first chunk's x and QK weights per-ct so the
            # first projection matmuls start after ~1/6 of the transfers
            for ct in range(nct):
                nc.sync.dma_start(wqkT_s[:, ct, :], wqk_r[:, ct, :])
                nc.sync.dma_start(xall[:, ct, 0:QB], xT_r[:, ct, 0:QB])
            nc.sync.dma_start(wvT_s[:], wvT_d.ap().rearrange("(ct p) d -> p ct d", p=128))
            nc.sync.dma_start(cst_s[:], cst_d.ap())
            def fetch_x(chp):
                if chp < n_ch:
                    for ct in range(nct):
                        nc.sync.dma_start(
                            xall[:, ct, chp * QB:(chp + 1) * QB],
                            xT_r[:, ct, chp * QB:(chp + 1) * QB])
            fetch_x(1)
            fetch_x(2)
            woT_a = const.tile([128, C], BF16)
            woT_b = const.tile([128, C], BF16)   # rows 64:128 are host zeros
            nc.sync.dma_start(woT_a[:], woT_d.ap()[0:128, :])
            nc.sync.dma_start(woT_b[:], woT_d.ap()[128:256, :])

            tri = cst_s[:, 0:128]       # 0 if p <= f else -1e30
            identb = cst_s[:, 128:256]
            onesb = cst_s[:, 256:352]

            ones1 = const.tile([128, 1], F32)
            nc.vector.memset(ones1[:], 1.0)
            zero1 = const.tile([128, 1], F32)
            nc.vector.memset(zero1[:], 0.0)

            # ---- big persistent activations ------------------------------
            KT01 = const.tile([128, t], BF16)
            KT2 = const.tile([128, t], BF16)
            QTz = [const.tile([128, t], BF16, tag=f"qtz{h}", name=f"qtz{h}")
                   for h in range(HPD)]
            Vone = const.tile([128, n_lt, HPD * 65], BF16)
            ctxT01 = const.tile([128, t], BF16)
            ctxT2 = const.tile([128, t], BF16)   # rows 64:128 zeroed

            # zero-fill dead rows (avoids NaN*0 in the PE); QTz/KT2 first —
            # attention qb=0 needs them
            for buf in (*QTz, KT2):
                nc.vector.tensor_copy(buf[:], zero1[:].to_broadcast((128, t)))
            nc.vector.tensor_copy(
                Vone[:].rearrange("p l (h e) -> p l h e", e=65)[:, :, :, 64:65],
                onesb[:])
            nc.vector.tensor_copy(
                ctxT2[64:128, :], zero1[0:64, :].to_broadcast((64, t)))

            def emit_outproj(qb, oc, pool):
                qs = slice(qb * QB, (qb + 1) * QB)
                ocs = slice(oc * 128, (oc + 1) * 128)
                po = pool.tile([128, 3 * QB] if pool is sp else [128, QB],
                               F32, tag="sp" if pool is sp else "po")
                nc.tensor.matmul(po[:, 0:QB], woT_a[:, ocs], ctxT01[:, qs],
                                 start=True, stop=False)
                nc.tensor.matmul(po[:, 0:QB], woT_b[:, ocs], ctxT2[:, qs],
                                 start=False, stop=True)
                ot = small.tile([128, QB], BF16, tag="ot")
                nc.vector.tensor_copy(ot[:], po[:, 0:QB])
                nc.sync.dma_start(outT_d.ap()[ocs, qs], ot[:])

            pending = []
            for ch in range(n_ch):
                cs = slice(ch * QB, (ch + 1) * QB)
                # ---- projection of chunk ch ------------------------------
                fetch_x(ch + 3)
                xc = xall[:, :, cs]

                t1 = sp.tile([128, 3 * QB], F32, tag="sp")
                for ct in range(nct):
                    f, l = (ct == 0), (ct == nct - 1)
                    nc.tensor.matmul(t1[:, 0:QB], wqkT_s[:, ct, 0:128],
                                     xc[:, ct, :], start=f, stop=l)
                    nc.tensor.matmul(t1[:, QB:2 * QB], wqkT_s[:, ct, 128:256],
                                     xc[:, ct, :], start=f, stop=l)
                    nc.tensor.matmul(t1[:, 2 * QB:3 * QB], wqkT_s[:, ct, 256:384],
                                     xc[:, ct, :], start=f, stop=l)
                t2 = sp.tile([128, 3 * QB], F32, tag="sp")
                for ts in range(QB // 128):
                    # 256-stride placement keeps each 192-col output inside
                    # one PSUM bank
                    pv = t2[:, ts * 256:ts * 256 + DH]
                    for ct in range(nct):
                        nc.tensor.matmul(pv, xc[:, ct, ts * 128:(ts + 1) * 128],
                                         wvT_s[:, ct, :], start=(ct == 0),
                                         stop=(ct == nct - 1))

                # copy projections out of PSUM (lane-aligned except K2)
                nc.vector.tensor_copy(QTz[0][0:64, cs], t1[0:64, 0:QB])
                nc.vector.tensor_copy(QTz[1][64:128, cs], t1[64:128, 0:QB])
                nc.vector.tensor_copy(KT01[:, cs], t1[:, QB:2 * QB])
                nc.vector.tensor_copy(QTz[2][0:64, cs], t1[0:64, 2 * QB:3 * QB])
                # K2 sits at partitions 64:128; stage + local DMA shifts it
                # down to KT2[0:64] (engines cannot move data across lanes)
                k2t = k2s.tile([128, QB], BF16, tag="k2t")
                nc.vector.tensor_copy(k2t[64:128, :], t1[64:128, 2 * QB:3 * QB])
                nc.sync.dma_start(KT2[0:64, cs], k2t[64:128, :])
                for ts in range(QB // 128):
                    tt = ch * (QB // 128) + ts
                    nc.vector.tensor_copy(
                        Vone[:, tt, :].rearrange("p (h e) -> p h e", e=65)[:, :, 0:64],
                        t2[:, ts * 256:ts * 256 + DH]
                          .rearrange("p (h e) -> p h e", e=64))

                # ---- attention q-block qb = ch ---------------------------
                qb = ch
                qs = cs
                L = (qb + 1) * ndg
                def normalize(h, ctxp):
                    # free the PSUM accumulator fast, normalize off-path
                    stg = small.tile([128, QB], F32, tag="stg")
                    nc.vector.tensor_copy(stg[0:65, :], ctxp[0:65, :])
                    dn = small.tile([1, QB], F32, tag="dn")
                    nc.vector.tensor_copy(dn[:], stg[64:65, :])
                    rec = small.tile([1, QB], F32, tag="rec")
                    nc.vector.reciprocal_approx_fast(rec[:], dn[:])
                    rb = small.tile([64, QB], F32, tag="rb")
                    nc.gpsimd.partition_broadcast(rb[:], rec[:])
                    if h == 1:
                        st2 = small.tile([64, QB], BF16, tag="st2")
                        nc.vector.tensor_mul(st2[:], stg[0:64, :], rb[:])
                        nc.sync.dma_start(ctxT01[64:128, qs], st2[:])
                    else:
                        dst = ctxT01 if h == 0 else ctxT2
                        nc.vector.tensor_mul(dst[0:64, qs], stg[0:64, :], rb[:])

                def emit_ctx(item):
                    h, g0, gl, et, ctxp, last = item
                    for i in range(gl):
                        lt = g0 + i
                        d = lt - qb * ndg
                        q0 = max(0, d) * LT
                        nc.tensor.matmul(ctxp[:, q0:QB],
                                         Vone[:, lt, h * 65:h * 65 + 65],
                                         et[:, i * QB + q0:(i + 1) * QB],
                                         start=(lt == 0), stop=(lt == L - 1),
                                         skip_group_check=True)
                    if last:
                        normalize(h, ctxp)

                # ctx groups are deferred one iteration so exp(g+1) never
                # transitively waits on ctx(g) through the PE completion
                # counter (the scores->exp->ctx lockstep in flat order)
                prev_ctx = None
                for h in range(HPD):
                    KT_h = KT01 if h < 2 else KT2
                    ctxp = cp.tile([65, QB], F32, tag="cp")
                    for g0 in range(0, L, GRP):
                        gl = min(GRP, L - g0)
                        spt = sp.tile([128, 3 * QB], F32, tag="sp")
                        for i in range(gl):
                            lt = g0 + i
                            d = lt - qb * ndg
                            kt = KT_h[:, lt * LT:(lt + 1) * LT]
                            if d < 0:
                                nc.tensor.matmul(spt[:, i * QB:(i + 1) * QB],
                                                 kt, QTz[h][:, qs],
                                                 start=True, stop=True)
                            else:
                                # cols 0..128d fully masked -> pruned.
                                # strip [q0, q0+128): triangle mask pre-acc
                                # + scores; beyond: plain scores.
                                q0 = d * LT
                                strip = spt[:, i * QB + q0:i * QB + q0 + LT]
                                nc.tensor.matmul(strip, identb, tri,
                                                 start=True, stop=False)
                                nc.tensor.matmul(
                                    strip, kt,
                                    QTz[h][:, qb * QB + q0:qb * QB + q0 + LT],
                                    start=False, stop=True)
                                if q0 + LT < QB:
                                    nc.tensor.matmul(
                                        spt[:, i * QB + q0 + LT:(i + 1) * QB],
                                        kt,
                                        QTz[h][:, qb * QB + q0 + LT:(qb + 1) * QB],
                                        start=True, stop=True)
                        et = epool.tile([128, GRP * QB], BF16)
                        nc.scalar.activation(et[:, :gl * QB], spt[:, :gl * QB],
                                             EXP, scale=0.125)
                        if prev_ctx is not None:
                            emit_ctx(prev_ctx)
                        prev_ctx = (h, g0, gl, et, ctxp, g0 + gl >= L)
                        if pending:
                            emit_outproj(*pending.pop(0), po_pool)
                emit_ctx(prev_ctx)
                pending.extend((qb, oc) for oc in range(nct))
            # tail: alternate the po bank and a free sp buffer so the last
            # six emissions overlap
            for n, item in enumerate(pending):
                emit_outproj(*item, po_pool if n % 2 == 0 else sp)

    nc.compile()
    return nc


_NC_CACHE = {}
LAST_EXEC_NS = None
LAST_RES = None


def _get_nc():
    if "full" not in _NC_CACHE:
        _NC_CACHE["full"] = build_kernel()
    return _NC_CACHE["full"]


def _install_ntff_shim():
    """Make run_bass_kernel_spmd(trace=True) work under axon in this image."""
    import antenv
    if "antenv.axon_hooks" in sys.modules:
        return
    mod = types.ModuleType("antenv.axon_hooks")
    mod._hook = None
    mod.set_axon_ntff_profile_hook = lambda h: setattr(mod, "_hook", h)
    mod.get_axon_ntff_profile_hook = lambda: mod._hook
    sys.modules["antenv.axon_hooks"] = mod
    antenv.axon_hooks = mod
    try:
        from trn_agent_boot.trn_boot import _ntff_profile_via_ctypes
        mod.set_axon_ntff_profile_hook(
            _ntff_profile_via_ctypes("/opt/axon/libaxon_pjrt.so"))
    except Exception:
        pass


def make_in_maps(x, wq, wk, wv, wo):
    x = np.asarray(x, dtype=np.float32)
    wq = np.asarray(wq, dtype=np.float32)
    wk = np.asarray(wk, dtype=np.float32)
    wv = np.asarray(wv, dtype=np.float32)
    wo = np.asarray(wo, dtype=np.float32)

    # constants: triangle mask, identity, ones
    p = np.arange(128)[:, None]
    f = np.arange(128)[None, :]
    tri = np.where(p <= f, 0.0, -1.0e30).astype(np.float32)
    cst = np.concatenate(
        [tri, np.eye(128, dtype=np.float32), np.ones((128, 96), np.float32)],
        axis=1).astype(BF)

    in_maps = []
    for c in range(NCORES):
        b, g = c // (NCORES // B), c % (NCORES // B)
        rs, re = g * DH, (g + 1) * DH
        # packed stationary: [wq01 | wk01 | wq2|wk2] (transposed)
        wqk = np.concatenate([
            wq[rs:rs + 128].T, wk[rs:rs + 128].T,
            wq[rs + 128:re].T, wk[rs + 128:re].T], axis=1)
        woT = np.zeros((256, C), dtype=np.float32)
        woT[:DH] = wo[:, rs:re].T
        in_maps.append({
            "xT": np.ascontiguousarray(x[b].T).astype(BF),
            "wqkT": np.ascontiguousarray(wqk).astype(BF),
            "wvT": np.ascontiguousarray(wv[rs:re].T).astype(BF),
            "woT": woT.astype(BF),
            "cst": cst,
        })
    return in_maps


def kernel(x, wq, wk, wv, wo):
    global LAST_EXEC_NS, LAST_RES
    in_maps = make_in_maps(x, wq, wk, wv, wo)
    nc = _get_nc()
    trace = bool(int(os.environ.get("KERNEL_TRACE", "0")))
    if trace:
        try:
            _install_ntff_shim()
        except Exception:
            trace = False
    try:
        res = run_bass_kernel_spmd(nc, in_maps, core_ids=list(range(NCORES)),
                                   trace=trace)
    except Exception:
        if not trace:
            raise
        res = run_bass_kernel_spmd(nc, in_maps, core_ids=list(range(NCORES)),
                                   trace=False)
    LAST_EXEC_NS = res.exec_time_ns
    LAST_RES = res
    outT = [res.results[c]["outT"] for c in range(NCORES)]
    halves = []
    for b in range(B):
        acc = outT[4 * b].astype(np.float64)
        for c in range(4 * b + 1, 4 * b + 4):
            acc = acc + outT[c]
        halves.append(acc.T)
    return np.stack(halves).astype(np.float32)


# revision 14
# speedup vs baseline: 1.1509x; 1.0972x over previous
"""Trainium2 Bass kernel: causal multi-head self-attention.

Problem: B=2, T=4096, C=768, H=12, D=64, causal softmax(QK^T/sqrt(D))V + out proj.

Sharding (8 cores): core c handles batch b=c//4 and 3 heads g=c%4 (rows
192*g:192*(g+1) of wq/wk/wv, same columns of wo). Each core computes its
heads' full attention and a partial out-projection (T, C) for its batch;
the host sums the 4 partials per batch and transposes back to (B, T, C).

v3 highlights (on top of the v2 proj/attention software pipeline):
  - x and all weights stream in as bf16 (half the DMA traffic; matmul rate
    on TRN2 is column-count-bound, so bf16 operands cost the same cycles
    but less bandwidth/weight-load time)
  - Q2/K2 projections merged into one matmul via a host-packed [wq2|wk2]
    stationary block; K2 lands at partitions 64:128 and is shifted to
    KT2[0:64] with a local SBUF->SBUF DMA (engines cannot cross partitions)
  - V projection at its true 192 columns (bf16 moving operand has no
    min-free-size penalty, unlike float32r)
  - diagonal column pruning: for the diagonal l-tile at depth d, columns
    0..128d of the q-block are fully masked, so the scores matmul, the mask
    pre-accumulation (one shared 128x128 triangle), and the ctx matmul all
    skip them. exp still covers the full group; the pruned garbage columns
    are never read.
  - masks/identity/ones constants precomputed on host, DMA'd once (the
    GpSimd affine_select chain used to serialize 20us of startup)
  - PSUM: sp pool 2x3 banks shared by projection tiles and score groups,
    1 bank ctx accumulator, 1 bank out-proj staging

Attention operands bf16, PSUM accumulation fp32, ctx/out in f32r.
"""

import os
import sys
import types

import numpy as np
import ml_dtypes

if "/opt/trn_rl_repo" not in sys.path:
    sys.path.insert(0, "/opt/trn_rl_repo")

import concourse.bass as bass  # noqa: E402
import concourse.mybir as mybir  # noqa: E402
from concourse import bacc, tile  # noqa: E402
from concourse.bass_utils import run_bass_kernel_spmd  # noqa: E402

F32 = mybir.dt.float32
F32R = mybir.dt.float32r
BF16 = mybir.dt.bfloat16
EXP = mybir.ActivationFunctionType.Exp

B, T, C, H, D = 2, 4096, 768, 12, 64
HPD = 3          # heads per device
DH = HPD * D     # 192 local head channels
NCORES = 8
QB = 512         # query block (matmul free dim / PSUM bank)
LT = 128         # key(l)-tile size
GRP = 3          # l-tiles per exp group (3 PSUM banks)
BF = ml_dtypes.bfloat16


def build_kernel(t=T, trace_sim=False):
    n_lt = t // LT
    n_ch = t // QB
    nct = C // 128            # 6
    ndg = QB // LT            # diagonal tiles per q-block (4)

    nc = bacc.Bacc("TRN2", target_bir_lowering=False, debug=False,
                   num_devices=NCORES)
    xT_d = nc.dram_tensor("xT", [C, t], BF16, kind="ExternalInput")
    # packed QK weights: [wq01(128) | wk01(128) | wq2(64)|wk2(64)]
    wqkT_d = nc.dram_tensor("wqkT", [C, 384], BF16, kind="ExternalInput")
    wvT_d = nc.dram_tensor("wvT", [C, DH], BF16, kind="ExternalInput")
    woT_d = nc.dram_tensor("woT", [256, C], BF16, kind="ExternalInput")  # padded
    # constants: [tri(128) | ident(128) | ones(96)]
    cst_d = nc.dram_tensor("cst", [128, 352], BF16, kind="ExternalInput")
    outT_d = nc.dram_tensor("outT", [C, t], BF16, kind="ExternalOutput")

    with tile.TileContext(nc, trace_sim=trace_sim) as tc:
        with (
            tc.tile_pool(name="const", bufs=1) as const,
            tc.tile_pool(name="epool", bufs=3) as epool,
            tc.tile_pool(name="small", bufs=3) as small,
            tc.tile_pool(name="k2s", bufs=2) as k2s,
            tc.tile_pool(name="sp", bufs=2, space="PSUM") as sp,
            tc.tile_pool(name="cp", bufs=1, space="PSUM") as cp,
            tc.tile_pool(name="fill", bufs=1, space="PSUM") as fill,
        ):
            # ---- x + weights: everything prefetched up front -------------
            # whole x lives in SBUF; per-(ct, chunk) DMAs give the
            # projection fine-grained dependencies while the DMA engines
            # stream the full tensor without per-chunk JIT pressure
            xall = const.tile([128, nct, t], BF16)
            xT_r = xT_d.ap().rearrange("(ct p) t -> p ct t", p=128)
            wqkT_s = const.tile([128, nct, 384], BF16)
            wvT_s = const.tile([128, nct, DH], BF16)
            cst_s = const.tile([128, 352], BF16)
            wqk_r = wqkT_d.ap().rearrange("(ct p) d -> p ct d", p=128)
            # interleave the first chunk's x and QK weights per-ct so the
            # first projection matmuls start after ~1/6 of the transfers
            for ct in range(nct):
                nc.sync.dma_start(wqkT_s[:, ct, :], wqk_r[:, ct, :])
                nc.sync.dma_start(xall[:, ct, 0:QB], xT_r[:, ct, 0:QB])
            nc.sync.dma_start(wvT_s[:], wvT_d.ap().rearrange("(ct p) d -> p ct d", p=128))
            nc.sync.dma_start(cst_s[:], cst_d.ap())
            def fetch_x(chp):
                if chp < n_ch:
                    for ct in range(nct):
                        nc.sync.dma_start(
                            xall[:, ct, chp * QB:(chp + 1) * QB],
                            xT_r[:, ct, chp * QB:(chp + 1) * QB])
            fetch_x(1)
            fetch_x(2)
            woT_a = const.tile([128, C], BF16)
            woT_b = const.tile([128, C], BF16)   # rows 64:128 are host zeros
            nc.sync.dma_start(woT_a[:], woT_d.ap()[0:128, :])
            nc.sync.dma_start(woT_b[:], woT_d.ap()[128:256, :])

            tri = cst_s[:, 0:128]       # 0 if p <= f else -1e30
            identb = cst_s[:, 128:256]
            onesb = cst_s[:, 256:352]

            ones1 = const.tile([128, 1], F32)
            nc.vector.memset(ones1[:], 1.0)
            zero1 = const.tile([128, 1], F32)
            nc.vector.memset(zero1[:], 0.0)

            # ---- big persistent activations ------------------------------
            KT01 = const.tile([128, t], BF16)
            KT2 = const.tile([128, t], BF16)
            QTz = [const.tile([128, t], BF16, tag=f"qtz{h}", name=f"qtz{h}")
                   for h in range(HPD)]
            Vone = const.tile([128, n_lt, HPD * 65], BF16)
            ctxT01 = const.tile([128, t], BF16)
            ctxT2 = const.tile([128, t], BF16)   # rows 64:128 zeroed

            # zero-fill dead rows (avoids NaN*0 in the PE); QTz/KT2 first —
            # attention qb=0 needs them
            for buf in (*QTz, KT2):
                nc.vector.tensor_copy(buf[:], zero1[:].to_broadcast((128, t)))
            nc.vector.tensor_copy(
                Vone[:].rearrange("p l (h e) -> p l h e", e=65)[:, :, :, 64:65],
                onesb[:])
            nc.vector.tensor_copy(
                ctxT2[64:128, :], zero1[0:64, :].to_broadcast((64, t)))

            def emit_outproj(qb, oc, pool):
                qs = slice(qb * QB, (qb + 1) * QB)
                ocs = slice(oc * 128, (oc + 1) * 128)
                po = pool.tile([128, 3 * QB] if pool is sp else [128, QB],
                               F32, tag="sp" if pool is sp else "fl")
                nc.tensor.matmul(po[:, 0:QB], woT_a[:, ocs], ctxT01[:, qs],
                                 start=True, stop=False)
                nc.tensor.matmul(po[:, 0:QB], woT_b[:, ocs], ctxT2[:, qs],
                                 start=False, stop=True)
                ot = small.tile([128, QB], BF16, tag="ot")
                nc.vector.tensor_copy(ot[:], po[:, 0:QB])
                nc.sync.dma_start(outT_d.ap()[ocs, qs], ot[:])

            def proj_pieces(ch):
                """Projection of chunk ch as 5 single-bank filler closures,
                interleaved between attention groups via the fill pool."""
                if ch >= n_ch:
                    return []
                cs = slice(ch * QB, (ch + 1) * QB)
                xc = xall[:, :, cs]

                def q01():
                    fp = fill.tile([128, QB], F32, tag="fl", name="fq01")
                    for ct in range(nct):
                        nc.tensor.matmul(fp[:], wqkT_s[:, ct, 0:128],
                                         xc[:, ct, :], start=(ct == 0),
                                         stop=(ct == nct - 1))
                    nc.vector.tensor_copy(QTz[0][0:64, cs], fp[0:64, :])
                    nc.vector.tensor_copy(QTz[1][64:128, cs], fp[64:128, :])

                def k01():
                    fp = fill.tile([128, QB], F32, tag="fl", name="fk01")
                    for ct in range(nct):
                        nc.tensor.matmul(fp[:], wqkT_s[:, ct, 128:256],
                                         xc[:, ct, :], start=(ct == 0),
                                         stop=(ct == nct - 1))
                    nc.vector.tensor_copy(KT01[:, cs], fp[:])

                def qk2():
                    fp = fill.tile([128, QB], F32, tag="fl", name="fqk2")
                    for ct in range(nct):
                        nc.tensor.matmul(fp[:], wqkT_s[:, ct, 256:384],
                                         xc[:, ct, :], start=(ct == 0),
                                         stop=(ct == nct - 1))
                    nc.vector.tensor_copy(QTz[2][0:64, cs], fp[0:64, :])
                    # K2 sits at partitions 64:128; stage + local DMA shifts
                    # it down to KT2[0:64]
                    k2t = k2s.tile([128, QB], BF16, tag="k2t")
                    nc.vector.tensor_copy(k2t[64:128, :], fp[64:128, :])
                    nc.sync.dma_start(KT2[0:64, cs], k2t[64:128, :])

                def vmk(tsbase):
                    def v2x():
                        fp = fill.tile([128, QB], F32, tag="fl", name="fv")
                        for tsi in range(2):
                            ts = tsbase + tsi
                            pv = fp[:, tsi * 256:tsi * 256 + DH]
                            for ct in range(nct):
                                nc.tensor.matmul(
                                    pv, xc[:, ct, ts * 128:(ts + 1) * 128],
                                    wvT_s[:, ct, :], start=(ct == 0),
                                    stop=(ct == nct - 1))
                        for tsi in range(2):
                            ts = tsbase + tsi
                            tt = ch * (QB // 128) + ts
                            nc.vector.tensor_copy(
                                Vone[:, tt, :].rearrange(
                                    "p (h e) -> p h e", e=65)[:, :, 0:64],
                                fp[:, tsi * 256:tsi * 256 + DH].rearrange(
                                    "p (h e) -> p h e", e=64))
                    return v2x

                return [q01, k01, qk2, vmk(0), vmk(2)]

            pending = []
            for f0 in proj_pieces(0):
                f0()
            fillers = []
            for ch in range(n_ch):
                cs = slice(ch * QB, (ch + 1) * QB)
                fetch_x(ch + 3)
                # next chunk's projection spreads across this q-block's
                # attention groups as single-bank filler pieces
                fillers = list(proj_pieces(ch + 1)) + fillers

                # ---- attention q-block qb = ch ---------------------------
                qb = ch
                qs = cs
                L = (qb + 1) * ndg
                def normalize(h, ctxp):
                    # free the PSUM accumulator fast, normalize off-path
                    stg = small.tile([128, QB], F32, tag="stg")
                    nc.vector.tensor_copy(stg[0:65, :], ctxp[0:65, :])
                    dn = small.tile([1, QB], F32, tag="dn")
                    nc.vector.tensor_copy(dn[:], stg[64:65, :])
                    rec = small.tile([1, QB], F32, tag="rec")
                    nc.vector.reciprocal_approx_fast(rec[:], dn[:])
                    rb = small.tile([64, QB], F32, tag="rb")
                    nc.gpsimd.partition_broadcast(rb[:], rec[:])
                    if h == 1:
                        st2 = small.tile([64, QB], BF16, tag="st2")
                        nc.vector.tensor_mul(st2[:], stg[0:64, :], rb[:])
                        nc.sync.dma_start(ctxT01[64:128, qs], st2[:])
                    else:
                        dst = ctxT01 if h == 0 else ctxT2
                        nc.vector.tensor_mul(dst[0:64, qs], stg[0:64, :], rb[:])

                def emit_ctx(item):
                    h, g0, gl, et, ctxp, last = item
                    for i in range(gl):
                        lt = g0 + i
                        d = lt - qb * ndg
                        q0 = max(0, d) * LT
                        nc.tensor.matmul(ctxp[:, q0:QB],
                                         Vone[:, lt, h * 65:h * 65 + 65],
                                         et[:, i * QB + q0:(i + 1) * QB],
                                         start=(lt == 0), stop=(lt == L - 1),
                                         skip_group_check=True)
                    if last:
                        normalize(h, ctxp)

                # ctx groups are deferred one iteration so exp(g+1) never
                # transitively waits on ctx(g) through the PE completion
                # counter (the scores->exp->ctx lockstep in flat order)
                prev_ctx = None
                for h in range(HPD):
                    KT_h = KT01 if h < 2 else KT2
                    ctxp = cp.tile([65, QB], F32, tag="cp")
                    for g0 in range(0, L, GRP):
                        gl = min(GRP, L - g0)
                        spt = sp.tile([128, 3 * QB], F32, tag="sp")
                        for i in range(gl):
                            lt = g0 + i
                            d = lt - qb * ndg
                            kt = KT_h[:, lt * LT:(lt + 1) * LT]
                            if d < 0:
                                nc.tensor.matmul(spt[:, i * QB:(i + 1) * QB],
                                                 kt, QTz[h][:, qs],
                                                 start=True, stop=True)
                            else:
                                # cols 0..128d fully masked -> pruned.
                                # strip [q0, q0+128): triangle mask pre-acc
                                # + scores; beyond: plain scores.
                                q0 = d * LT
                                strip = spt[:, i * QB + q0:i * QB + q0 + LT]
                                nc.tensor.matmul(strip, identb, tri,
                                                 start=True, stop=False)
                                nc.tensor.matmul(
                                    strip, kt,
                                    QTz[h][:, qb * QB + q0:qb * QB + q0 + LT],
                                    start=False, stop=True)
                                if q0 + LT < QB:
                                    nc.tensor.matmul(
                                        spt[:, i * QB + q0 + LT:(i + 1) * QB],
                                        kt,
                                        QTz[h][:, qb * QB + q0 + LT:(qb + 1) * QB],
                                        start=True, stop=True)
                        et = epool.tile([128, GRP * QB], BF16)
                        nc.scalar.activation(et[:, :gl * QB], spt[:, :gl * QB],
                                             EXP, scale=0.125)
                        if prev_ctx is not None:
                            emit_ctx(prev_ctx)
                        prev_ctx = (h, g0, gl, et, ctxp, g0 + gl >= L)
                        if fillers:
                            fillers.pop(0)()
                        elif pending:
                            emit_outproj(*pending.pop(0), fill)
                emit_ctx(prev_ctx)
                # next chunk's projection pieces must finish this iteration
                for fn in fillers:
                    fn()
                fillers = []
                pending.extend((qb, oc) for oc in range(nct))
            # tail: alternate the fill bank and a free sp buffer so the
            # last emissions overlap
            for n, item in enumerate(pending):
                emit_outproj(*item, fill if n % 2 == 0 else sp)

    nc.compile()
    return nc


_NC_CACHE = {}
LAST_EXEC_NS = None
LAST_RES = None


def _get_nc():
    if "full" not in _NC_CACHE:
        _NC_CACHE["full"] = build_kernel()
    return _NC_CACHE["full"]


def _install_ntff_shim():
    """Make run_bass_kernel_spmd(trace=True) work under axon in this image."""
    import antenv
    if "antenv.axon_hooks" in sys.modules:
        return
    mod = types.ModuleType("antenv.axon_hooks")
    mod._hook = None
    mod.set_axon_ntff_profile_hook = lambda h: setattr(mod, "_hook", h)
    mod.get_axon_ntff_profile_hook = lambda: mod._hook
    sys.modules["antenv.axon_hooks"] = mod
    antenv.axon_hooks = mod
    try:
        from trn_agent_boot.trn_boot import _ntff_profile_via_ctypes
        mod.set_axon_ntff_profile_hook(
            _ntff_profile_via_ctypes("/opt/axon/libaxon_pjrt.so"))
    except Exception:
        pass


def make_in_maps(x, wq, wk, wv, wo):
    x = np.asarray(x, dtype=np.float32)
    wq = np.asarray(wq, dtype=np.float32)
    wk = np.asarray(wk, dtype=np.float32)
    wv = np.asarray(wv, dtype=np.float32)
    wo = np.asarray(wo, dtype=np.float32)

    # constants: triangle mask, identity, ones
    p = np.arange(128)[:, None]
    f = np.arange(128)[None, :]
    tri = np.where(p <= f, 0.0, -1.0e30).astype(np.float32)
    cst = np.concatenate(
        [tri, np.eye(128, dtype=np.float32), np.ones((128, 96), np.float32)],
        axis=1).astype(BF)

    in_maps = []
    for c in range(NCORES):
        b, g = c // (NCORES // B), c % (NCORES // B)
        rs, re = g * DH, (g + 1) * DH
        # packed stationary: [wq01 | wk01 | wq2|wk2] (transposed)
        wqk = np.concatenate([
            wq[rs:rs + 128].T, wk[rs:rs + 128].T,
            wq[rs + 128:re].T, wk[rs + 128:re].T], axis=1)
        woT = np.zeros((256, C), dtype=np.float32)
        woT[:DH] = wo[:, rs:re].T
        in_maps.append({
            "xT": np.ascontiguousarray(x[b].T).astype(BF),
            "wqkT": np.ascontiguousarray(wqk).astype(BF),
            "wvT": np.ascontiguousarray(wv[rs:re].T).astype(BF),
            "woT": woT.astype(BF),
            "cst": cst,
        })
    return in_maps


def kernel(x, wq, wk, wv, wo):
    global LAST_EXEC_NS, LAST_RES
    in_maps = make_in_maps(x, wq, wk, wv, wo)
    nc = _get_nc()
    trace = bool(int(os.environ.get("KERNEL_TRACE", "0")))
    if trace:
        try:
            _install_ntff_shim()
        except Exception:
            trace = False
    try:
        res = run_bass_kernel_spmd(nc, in_maps, core_ids=list(range(NCORES)),
                                   trace=trace)
    except Exception:
        if not trace:
            raise
        res = run_bass_kernel_spmd(nc, in_maps, core_ids=list(range(NCORES)),
                                   trace=False)
    LAST_EXEC_NS = res.exec_time_ns
    LAST_RES = res
    outT = [res.results[c]["outT"] for c in range(NCORES)]
    halves = []
    for b in range(B):
        acc = outT[4 * b].astype(np.float64)
        for c in range(4 * b + 1, 4 * b + 4):
            acc = acc + outT[c]
        halves.append(acc.T)
    return np.stack(halves).astype(np.float32)
